# revision 9
# baseline (speedup 1.0000x reference)
"""Trainium2 Bass kernel for nn_Decoder (Bahdanau attention + 3-layer GRU decoder + BN).

Strategy:
  - Attention: data-parallel over batch (16 rows/core). enc_output is
    host-pre-transposed to [H, B_loc, S] so the H-contraction lands on SBUF
    partitions; main matmul Y^T = w1^T @ encT runs in float32r (full PE rate).
    Bias + tanh fuse into one ScalarE activation per tile (bias is
    per-partition in the transposed layout). score = vw^T @ tanh(Y^T) as bf16
    rank-1 matmuls. Softmax skips max-subtraction (|score| <= ~16 bounded by
    tanh in [-1,1] and small vw), so exp-weighted context accumulates in the
    same single pass over enc.
  - GRU + BatchNorm: tensor-parallel over hidden (128 cols/core); 3 AllGathers
    move context/hidden between phases. BN batch stats are local since every
    core holds all 128 batch rows in the TP layout.
"""
import numpy as np
import ml_dtypes

import concourse.bass as bass
import concourse.bacc as bacc
import concourse.tile as tile
import concourse.mybir as mybir
from concourse.bass_utils import run_bass_kernel_spmd

F32 = mybir.dt.float32
F32R = mybir.dt.float32r
BF16 = mybir.dt.bfloat16
AF = mybir.ActivationFunctionType
ALU = mybir.AluOpType

B, S, H, E, V, NL = 128, 1024, 1024, 512, 10000, 3
BN_EPS = 1e-3
NCORES = 8
BL = B // NCORES          # 16 local batch rows per core (attention)
KT = H // 128             # 8 k-tiles of H
HHALF = S // 2            # 512, the r_tile free size
IN0 = H + E               # 1536 = GRU layer-0 input width
NK0 = IN0 // 128          # 12

_CACHE = {}


def _build():
    if "nc" in _CACHE:
        return _CACHE["nc"]
    nc = bacc.Bacc(None, target_bir_lowering=False)

    # ---------------- DRAM I/O ----------------
    encT_d = nc.dram_tensor("encT", [H, BL, S], F32R, kind="ExternalInput")
    w1_d = nc.dram_tensor("w1", [H, H], F32R, kind="ExternalInput")
    w2_d = nc.dram_tensor("w2", [H, H], F32, kind="ExternalInput")
    hsT_d = nc.dram_tensor("hsT", [128, KT * BL], F32, kind="ExternalInput")
    b12_d = nc.dram_tensor("b12", [128, KT], F32, kind="ExternalInput")
    vw_d = nc.dram_tensor("vwT", [128, KT], BF16, kind="ExternalInput")
    xeT_d = nc.dram_tensor("xeT", [E, B], F32, kind="ExternalInput")

    kg_d = []   # per layer: [kz, kr, kh] each [in_dim, 128]
    rkg_d = []  # per layer (l>=1): [rkz, rkr, rkh] each [H, 128]
    aux_d = []  # per layer: [128, 6] = bz, br, b0h, b1h, gamma, beta
    in_dims = [IN0, H, H]
    for l in range(NL):
        kg_d.append([
            nc.dram_tensor(f"k{l}{g}", [in_dims[l], 128], F32, kind="ExternalInput")
            for g in "zrh"
        ])
        if l >= 1:
            rkg_d.append([
                nc.dram_tensor(f"rk{l}{g}", [H, 128], F32, kind="ExternalInput")
                for g in "zrh"
            ])
        else:
            rkg_d.append(None)
        aux_d.append(nc.dram_tensor(f"aux{l}", [128, 6], F32, kind="ExternalInput"))

    attn_d = nc.dram_tensor("attn", [1, BL * S], F32, kind="ExternalOutput")
    y2_d = nc.dram_tensor("y2s", [128, B], F32, kind="ExternalOutput")
    h2_d = nc.dram_tensor("h2s", [128, B], F32, kind="ExternalOutput")

    with tile.TileContext(nc) as tc:
        with (
            tc.tile_pool(name="persist", bufs=1) as persist,
            tc.tile_pool(name="dram", bufs=1, space="DRAM") as dram,
        ):
            # ---- persistent SBUF state ----
            w1_sb = persist.tile([128, KT, H], F32R, tag="w1")    # [p, kc, j]
            nc.sync.dma_start(w1_sb[:], w1_d.rearrange("(kc p) j -> p kc j", p=128))
            vw_sb = persist.tile([128, KT], BF16, tag="vw")
            nc.sync.dma_start(vw_sb[:], vw_d[:])
            b12_sb = persist.tile([128, KT], F32, tag="b12")
            nc.sync.dma_start(b12_sb[:], b12_d[:])
            hsT_sb = persist.tile([128, KT * BL], F32, tag="hsT")
            nc.sync.dma_start(hsT_sb[:], hsT_d[:])

            ones_sb = persist.tile([1, 128], F32, tag="ones")
            nc.vector.memset(ones_sb[:], 1.0)
            eps_sb = persist.tile([128, 1], F32, tag="eps")
            nc.vector.memset(eps_sb[:], BN_EPS)

            cT_sb = persist.tile([128, KT * BL], F32, tag="cT")   # attn bias c[b,j]
            E_sb = persist.tile([1, BL * S], F32, tag="E")        # exp(score) rows
            Zp_sb = persist.tile([1, 2 * BL], F32, tag="Zp")      # per-r_tile Z parts
            acc_sb = persist.tile([128, BL * KT], F32, tag="acc")  # ctx accumulator
            nc.vector.memset(acc_sb[:], 0.0)

            # ---- q = hs @ w2 + (b1+b2), in transposed layout cT[j, (jt,l)] ----
            with (
                tc.tile_pool(name="w2p", bufs=1) as w2p,
                tc.tile_pool(name="qps", bufs=2, space="PSUM") as qps,
            ):
                w2_sb = w2p.tile([128, KT, H], F32, tag="w2")
                nc.sync.dma_start(w2_sb[:], w2_d.rearrange("(kc p) j -> p kc j", p=128))
                for jt in range(KT):
                    ps_q = qps.tile([128, BL], F32, tag="psq")
                    for kc in range(KT):
                        nc.tensor.matmul(
                            ps_q[:],
                            w2_sb[:, kc, jt * 128:(jt + 1) * 128],
                            hsT_sb[:, kc * BL:(kc + 1) * BL],
                            start=(kc == 0), stop=(kc == KT - 1),
                        )
                    nc.vector.tensor_scalar(
                        cT_sb[:, jt * BL:(jt + 1) * BL], ps_q[:],
                        b12_sb[:, jt:jt + 1], None, ALU.add,
                    )

            # ---- attention main loop ----
            with (
                tc.tile_pool(name="enc", bufs=3) as encp,
                tc.tile_pool(name="tanh", bufs=3) as tanhp,
                tc.tile_pool(name="ctx", bufs=2) as ctxp,
                tc.tile_pool(name="mm", bufs=2, space="PSUM") as mmp,
                tc.tile_pool(name="bc", bufs=2, space="PSUM") as bcp,
                tc.tile_pool(name="sc", bufs=2, space="PSUM") as scp,
                tc.tile_pool(name="rc", bufs=1, space="PSUM") as rcp,
            ):
                enc_v = encT_d.rearrange("(kc p) b (h s) -> b h p kc s", p=128, h=2)
                for l in range(BL):
                    for half in range(2):
                        r_idx = l * 2 + half
                        pos = l * S + half * HHALF
                        enc_sb = encp.tile([128, KT, HHALF], F32R, tag="enc")
                        nc.sync.dma_start(enc_sb[:], enc_v[l, half])

                        score_ps = scp.tile([1, HHALF], F32, tag="score")
                        for jt in range(KT):
                            ps_y = mmp.tile([128, HHALF], F32, tag="psy")
                            for kc in range(KT):
                                nc.tensor.matmul(
                                    ps_y[:],
                                    w1_sb[:, kc, jt * 128:(jt + 1) * 128],
                                    enc_sb[:, kc, :],
                                    start=(kc == 0), stop=(kc == KT - 1),
                                )
                            tanh_sb = tanhp.tile([128, HHALF], BF16, tag="tanh")
                            nc.scalar.activation(
                                tanh_sb[:], ps_y[:], AF.Tanh,
                                bias=cT_sb[:, jt * BL + l: jt * BL + l + 1],
                            )
                            nc.tensor.matmul(
                                score_ps[:], vw_sb[:, jt:jt + 1], tanh_sb[:],
                                start=(jt == 0), stop=(jt == KT - 1),
                            )
                        # E = exp(score); Z part via fused accumulation
                        nc.scalar.activation(
                            E_sb[0:1, pos:pos + HHALF], score_ps[:], AF.Exp,
                            accum_out=Zp_sb[0:1, r_idx:r_idx + 1],
                        )
                        # broadcast E across partitions: ones^T @ E_row
                        bc_ps = bcp.tile([128, HHALF], F32, tag="bc")
                        nc.tensor.matmul(
                            bc_ps[:], ones_sb[:], E_sb[0:1, pos:pos + HHALF],
                            start=True, stop=True,
                        )
                        # context: ctxU[j] += sum_s encT[j,s] * E[s]
                        tmp8 = ctxp.tile([128, KT], F32, tag="tmp8")
                        for kc in range(KT):
                            prod = ctxp.tile([128, HHALF], F32, tag="prod")
                            nc.vector.tensor_tensor(
                                prod[:], enc_sb[:, kc, :].bitcast(F32), bc_ps[:],
                                ALU.mult,
                            )
                            nc.vector.tensor_reduce(
                                tmp8[:, kc:kc + 1], prod[:], mybir.AxisListType.X,
                                ALU.add,
                            )
                        nc.vector.tensor_tensor(
                            acc_sb[:, l * KT:(l + 1) * KT],
                            acc_sb[:, l * KT:(l + 1) * KT], tmp8[:], ALU.add,
                        )
                    # per-l normalization: r_l = 1/(Zp[2l]+Zp[2l+1])
                    zsum = ctxp.tile([1, 1], F32, tag="zsum")
                    nc.vector.tensor_reduce(
                        zsum[:], Zp_sb[0:1, 2 * l:2 * l + 2],
                        mybir.AxisListType.X, ALU.add,
                    )
                    rl = ctxp.tile([1, 1], F32, tag="rl")
                    nc.vector.reciprocal(rl[:], zsum[:])
                    # attn row: E[l*S:(l+1)*S] *= r_l  (in place)
                    nc.vector.tensor_scalar(
                        E_sb[0:1, l * S:(l + 1) * S], E_sb[0:1, l * S:(l + 1) * S],
                        rl[:], None, ALU.mult,
                    )
                    # ctx slice *= r_l : broadcast r_l to all partitions via PE
                    rcol_ps = rcp.tile([128, 1], F32, tag="rcol")
                    nc.tensor.matmul(rcol_ps[:], ones_sb[:], rl[:], start=True, stop=True)
                    rcol = ctxp.tile([128, 1], F32, tag="rcol_sb")
                    nc.vector.tensor_copy(rcol[:], rcol_ps[:])
                    nc.vector.tensor_scalar(
                        acc_sb[:, l * KT:(l + 1) * KT], acc_sb[:, l * KT:(l + 1) * KT],
                        rcol[:], None, ALU.mult,
                    )

            # attn output: E_sb holds normalized weights in [l, s] flat order
            nc.sync.dma_start(attn_d[:], E_sb[:])

            # ---- AllGather context ----
            ctxC = dram.tile([H, BL], F32, tag="ctxC")
            ctxC_v = ctxC.rearrange("(jt p) l -> jt p l", p=128)
            acc_v = acc_sb.rearrange("p (l jt) -> jt p l", jt=KT)
            for jt in range(KT):
                nc.sync.dma_start(ctxC_v[jt], acc_v[jt])
            ctxG = dram.tile([NCORES, H, BL], F32, tag="ctxG")
            nc.gpsimd.collective_compute(
                "AllGather", ALU.bypass,
                replica_groups=[list(range(NCORES))],
                ins=[ctxC.opt()], outs=[ctxG.opt()],
            )

            # ---- GRU phase (tensor-parallel over hidden) ----
            with tc.tile_pool(name="xt", bufs=1) as xtp:
                # xtT tiles: [128, B] per k-tile; 8 ctx tiles + 4 xe tiles
                xt_tiles = []
                ctxg_v = ctxG.rearrange("r (kt p) l -> kt p r l", p=128)
                for kt in range(KT):
                    t = xtp.tile([128, B], F32, tag=f"xt{kt}")
                    nc.sync.dma_start(
                        t[:].rearrange("p (r l) -> p r l", r=NCORES), ctxg_v[kt])
                    xt_tiles.append(t)
                xe_v = xeT_d.rearrange("(kt p) b -> kt p b", p=128)
                for kt in range(E // 128):
                    t = xtp.tile([128, B], F32, tag=f"xte{kt}")
                    nc.sync.dma_start(t[:], xe_v[kt])
                    xt_tiles.append(t)

                h_slice_prev = None   # this core's h_{l-1} slice [128, B]
                in_tiles = xt_tiles
                for l in range(NL):
                    nk = len(in_tiles)
                    with (
                        tc.tile_pool(name=f"gw{l}", bufs=1) as gw,
                        tc.tile_pool(name=f"gs{l}", bufs=1) as gs,
                        tc.tile_pool(name=f"gp{l}", bufs=1, space="PSUM") as gp,
                    ):
                        aux_sb = gs.tile([128, 6], F32, tag="aux")
                        nc.sync.dma_start(aux_sb[:], aux_d[l][:])
                        bz, br, b0h, b1h, gam, bet = (aux_sb[:, i:i + 1] for i in range(6))

                        k_sb = []
                        for gi, g in enumerate("zrh"):
                            t = gw.tile([128, nk, 128], F32, tag=f"k{g}")
                            nc.sync.dma_start(
                                t[:], kg_d[l][gi].rearrange("(kt p) m -> p kt m", p=128))
                            k_sb.append(t)
                        rk_sb = []
                        if l >= 1:
                            for gi, g in enumerate("zrh"):
                                t = gw.tile([128, KT, 128], F32, tag=f"rk{g}")
                                nc.sync.dma_start(
                                    t[:], rkg_d[l][gi].rearrange("(kt p) m -> p kt m", p=128))
                                rk_sb.append(t)

                        # gate psums: z and r accumulate input+recurrent together
                        ps = []
                        for gi in range(3):
                            p = gp.tile([128, B], F32, tag=f"ps{gi}")
                            for kt in range(nk):
                                nc.tensor.matmul(
                                    p[:], k_sb[gi][:, kt, :], in_tiles[kt][:],
                                    start=(kt == 0), stop=(l == 0 or gi == 2) and (kt == nk - 1),
                                )
                            ps.append(p)
                        ps_ih = None
                        if l >= 1:
                            for gi in range(2):   # z, r recurrent into same psum
                                for kt in range(KT):
                                    nc.tensor.matmul(
                                        ps[gi][:], rk_sb[gi][:, kt, :], hprev_tiles[kt][:],
                                        start=False, stop=(kt == KT - 1),
                                    )
                            ps_ih = gp.tile([128, B], F32, tag="psih")
                            for kt in range(KT):
                                nc.tensor.matmul(
                                    ps_ih[:], rk_sb[2][:, kt, :], hprev_tiles[kt][:],
                                    start=(kt == 0), stop=(kt == KT - 1),
                                )

                        z_sb = gs.tile([128, B], F32, tag="z")
                        nc.scalar.activation(z_sb[:], ps[0][:], AF.Sigmoid, bias=bz)
                        r_sb = gs.tile([128, B], F32, tag="r")
                        nc.scalar.activation(r_sb[:], ps[1][:], AF.Sigmoid, bias=br)

                        rih = gs.tile([128, B], F32, tag="rih")
                        if l == 0:
                            nc.vector.tensor_scalar(rih[:], r_sb[:], b1h, None, ALU.mult)
                        else:
                            ih_sb = gs.tile([128, B], F32, tag="ih")
                            nc.vector.tensor_scalar(ih_sb[:], ps_ih[:], b1h, None, ALU.add)
                            nc.vector.tensor_tensor(rih[:], r_sb[:], ih_sb[:], ALU.mult)
                        hpre = gs.tile([128, B], F32, tag="hpre")
                        nc.vector.tensor_tensor(hpre[:], rih[:], ps[2][:], ALU.add)
                        hh = gs.tile([128, B], F32, tag="hh")
                        act_fn = AF.Relu if l < NL - 1 else AF.Identity
                        nc.scalar.activation(hh[:], hpre[:], act_fn, bias=b0h)

                        h_new = xtp.tile([128, B], F32, tag=f"hnew{l}")
                        t1 = gs.tile([128, B], F32, tag="t1")
                        if l == 0:
                            nc.vector.tensor_tensor(t1[:], z_sb[:], hh[:], ALU.mult)
                            nc.vector.tensor_tensor(h_new[:], hh[:], t1[:], ALU.subtract)
                        else:
                            d = gs.tile([128, B], F32, tag="d")
                            nc.vector.tensor_tensor(d[:], h_slice_prev[:], hh[:], ALU.subtract)
                            nc.vector.tensor_tensor(t1[:], z_sb[:], d[:], ALU.mult)
                            nc.vector.tensor_tensor(h_new[:], hh[:], t1[:], ALU.add)

                        # BatchNorm over batch (free dim), training stats
                        s1 = gs.tile([128, 1], F32, tag="s1")
                        nc.vector.tensor_reduce(s1[:], h_new[:], mybir.AxisListType.X, ALU.add)
                        mu = gs.tile([128, 1], F32, tag="mu")
                        nc.vector.tensor_scalar(mu[:], s1[:], 1.0 / B, None, ALU.mult)
                        hsq = gs.tile([128, B], F32, tag="hsq")
                        nc.vector.tensor_tensor(hsq[:], h_new[:], h_new[:], ALU.mult)
                        s2 = gs.tile([128, 1], F32, tag="s2")
                        nc.vector.tensor_reduce(s2[:], hsq[:], mybir.AxisListType.X, ALU.add)
                        m2 = gs.tile([128, 1], F32, tag="m2")
                        nc.vector.tensor_scalar(m2[:], s2[:], 1.0 / B, None, ALU.mult)
                        mu2 = gs.tile([128, 1], F32, tag="mu2")
                        nc.vector.tensor_tensor(mu2[:], mu[:], mu[:], ALU.mult)
                        var = gs.tile([128, 1], F32, tag="var")
                        nc.vector.tensor_tensor(var[:], m2[:], mu2[:], ALU.subtract)
                        sd = gs.tile([128, 1], F32, tag="sd")
                        nc.scalar.activation(sd[:], var[:], AF.Sqrt, bias=eps_sb[:])
                        inv = gs.tile([128, 1], F32, tag="inv")
                        nc.vector.reciprocal(inv[:], sd[:])
                        sc = gs.tile([128, 1], F32, tag="sc")
                        nc.vector.tensor_tensor(sc[:], gam, inv[:], ALU.mult)
                        scmu = gs.tile([128, 1], F32, tag="scmu")
                        nc.vector.tensor_tensor(scmu[:], sc[:], mu[:], ALU.mult)
                        sh = gs.tile([128, 1], F32, tag="sh")
                        nc.vector.tensor_tensor(sh[:], bet, scmu[:], ALU.subtract)
                        y_sb = gs.tile([128, B], F32, tag="y")
                        nc.vector.tensor_scalar(y_sb[:], h_new[:], sc[:], sh[:],
                                                ALU.mult, ALU.add)

                        if l < NL - 1:
                            # AllGather y and h for next layer
                            ygC = dram.tile([2, 128, B], F32, tag=f"ygC{l}")
                            nc.sync.dma_start(ygC[0], y_sb[:])
                            nc.sync.dma_start(ygC[1], h_new[:])
                            ygG = dram.tile([NCORES, 2, 128, B], F32, tag=f"ygG{l}")
                            nc.gpsimd.collective_compute(
                                "AllGather", ALU.bypass,
                                replica_groups=[list(range(NCORES))],
                                ins=[ygC.opt()], outs=[ygG.opt()],
                            )
                            y_tiles, h_tiles = [], []
                            for kt in range(KT):
                                ty = xtp.tile([128, B], F32, tag=f"y{l}_{kt}")
                                nc.sync.dma_start(ty[:], ygG[kt, 0])
                                y_tiles.append(ty)
                                th = xtp.tile([128, B], F32, tag=f"h{l}_{kt}")
                                nc.sync.dma_start(th[:], ygG[kt, 1])
                                h_tiles.append(th)
                            in_tiles = y_tiles
                            hprev_tiles = h_tiles
                            h_slice_prev = h_new
                        else:
                            nc.sync.dma_start(y2_d[:], y_sb[:])
                            nc.sync.dma_start(h2_d[:], h_new[:])

    nc.compile()
    _CACHE["nc"] = nc
    return nc


def _prep_inputs(inputs):
    """Host-side layout prep: slice/transpose/gather. Returns per-core in_maps."""
    g = {k: np.asarray(v) for k, v in inputs.items()}
    enc = np.asarray(g["enc_output"], dtype=np.float32)
    encT_full = np.ascontiguousarray(enc.transpose(2, 0, 1))   # [H, B, S]
    w1 = np.asarray(g["w1"], np.float32)
    w2 = np.asarray(g["w2"], np.float32)
    b12 = (np.asarray(g["b1"], np.float32) + np.asarray(g["b2"], np.float32))
    b12_t = np.ascontiguousarray(b12.reshape(KT, 128).T)
    vw_t = np.ascontiguousarray(
        np.asarray(g["vw"], np.float32)[:, 0].reshape(KT, 128).T
    ).astype(ml_dtypes.bfloat16)
    hs = np.asarray(g["hidden_state"], np.float32)
    emb = np.asarray(g["emb"], np.float32)
    x = np.asarray(g["x"]).astype(np.int64)[:, 0]
    xeT = np.ascontiguousarray(emb[x].T)                        # [E, B]

    in_dims = [IN0, H, H]
    in_maps = []
    for c in range(NCORES):
        bs = slice(c * BL, (c + 1) * BL)
        ci = slice(c * 128, (c + 1) * 128)
        m = {
            "encT": np.ascontiguousarray(encT_full[:, bs, :]),
            "w1": w1,
            "w2": w2,
            "hsT": np.ascontiguousarray(
                hs[bs].T.reshape(KT, 128, BL).transpose(1, 0, 2).reshape(128, KT * BL)),
            "b12": b12_t,
            "vwT": vw_t,
            "xeT": xeT,
        }
        for l in range(NL):
            k = np.asarray(g[f"k{l}"], np.float32)
            rk = np.asarray(g[f"rk{l}"], np.float32)
            bias = np.asarray(g[f"bias{l}"], np.float32)
            gam = np.asarray(g[f"gamma{l}"], np.float32)
            bet = np.asarray(g[f"beta{l}"], np.float32)
            for gi, gname in enumerate("zrh"):
                col = slice(gi * H + c * 128, gi * H + c * 128 + 128)
                m[f"k{l}{gname}"] = np.ascontiguousarray(k[:, col])
                if l >= 1:
                    m[f"rk{l}{gname}"] = np.ascontiguousarray(rk[:, col])
            bz = bias[0, ci] + bias[1, ci]
            br = bias[0, H + c * 128: H + c * 128 + 128] + bias[1, H + c * 128: H + c * 128 + 128]
            b0h = bias[0, 2 * H + c * 128: 2 * H + c * 128 + 128]
            b1h = bias[1, 2 * H + c * 128: 2 * H + c * 128 + 128]
            m[f"aux{l}"] = np.ascontiguousarray(
                np.stack([bz, br, b0h, b1h, gam[ci], bet[ci]], axis=1))
        in_maps.append(m)
    return in_maps


def kernel(**inputs):
    nc = _build()
    in_maps = _prep_inputs(inputs)
    res = run_bass_kernel_spmd(nc, in_maps, core_ids=list(range(NCORES)))
    results = res.results

    attn = np.concatenate(
        [results[c]["attn"].reshape(BL, S) for c in range(NCORES)], axis=0)
    attn_w = attn.reshape(B, S, 1).astype(np.float32)
    y2 = np.concatenate([results[c]["y2s"] for c in range(NCORES)], axis=0).T
    h2 = np.concatenate([results[c]["h2s"] for c in range(NCORES)], axis=0).T
    return (np.ascontiguousarray(y2.astype(np.float32)),
            np.ascontiguousarray(h2.astype(np.float32)),
            attn_w)
